# revision 4
# baseline (speedup 1.0000x reference)
"""Trainium2 Bass kernel for nn_Attention_v4 (sparse per-atom attention).

Reference computation (fp32):
    x:[2,512,14,1024] -> qkv = x@w_qkv+b_qkv -> per (b, r=atom, head) attention
    over the n=512 axis -> out @ w_proj + b_proj.

Sharding (8 cores): 4 groups x 7 (b,r)-units data-parallel, x 2 head-halves
tensor-parallel. Each core computes, for its 7 units and its 8 heads:
QKV^T projection, attention, and a partial c_proj (contraction over its 512
of the 1024 hd rows). Host unshard sums the two head-half partials (the
"all-reduce" of the TP split) and adds b_proj.

Device layouts (all matmuls in float32r: full PE rate at N>=256):
  qkT  [col(q 512|k 512), tok]  - from lhsT=w tiles (stationary), rhs=xT
  v    [tok, lh*65+d]           - 65th col per head = 1.0 (via zero weight
                                  col + bias 1) -> P@V also yields softmax
                                  denominators as row 64 of each head block
  S^T  [j, i] = kT.T @ qT       - softmax over j = partition dim; scores are
                                  O(3) so exp without max-subtraction is safe
  O^T  [hd, i] = (v' @ expS) / denom
  out  [tok, e] partial = sum_ct OT_ct.T @ wproj_ct

v2 pipeline notes (HW-probe driven):
  - scores for a head PAIR share one [128,1024] psum tile and ONE exp, so
    the ACT round-trip is paid once per pair-step and the 2-buf psum ring
    gives ~4 head-steps of PE run-ahead (v1 stalled ~1.4us per head-step).
  - psum: ps_big 2 bufs x 2 banks (qk/v/st/c_proj chunks), ps_o 4 bufs
    (two head-pairs of attention output in flight) = 8 banks exactly.
  - psum drains ride on ACT (Copy/Identity+bias); DVE keeps v-bias adds
    and softmax normalize; GPSIMD only broadcasts the recip denominators.
  - per-pair ot tiles let c_proj's ct-chunk start as soon as that pair is
    normalized instead of waiting for all 8 heads.
"""

import numpy as np

B, N, A, DIM, H, D = 2, 512, 14, 1024, 16, 64
HL = 8            # heads per core
UNITS = 7         # (b, r) units per group
NCORES = 8
SCALE = np.float32(1.0 / np.sqrt(np.sqrt(D)))
VW = D + 1        # v width per head incl. ones column

_CACHE = {}


def _build_nc(units=UNITS, repeat=1, phases="QAC", qk_bias=False):
    import concourse.bacc as bacc
    import concourse.tile as tile
    from concourse import mybir
    from concourse.bass import ts

    f32, f32r = mybir.dt.float32, mybir.dt.float32r
    AF = mybir.ActivationFunctionType

    nc = bacc.Bacc("TRN2", target_bir_lowering=False, debug=False,
                   num_devices=NCORES)
    xT = nc.dram_tensor("xT", [units, DIM, N], f32r, kind="ExternalInput")
    wqkv = nc.dram_tensor("wqkv", [DIM, 1024 + HL * D], f32r,
                          kind="ExternalInput")
    bqk = nc.dram_tensor("bqk", [1024], f32, kind="ExternalInput")
    bv = nc.dram_tensor("bv", [HL * VW], f32, kind="ExternalInput")
    wproj = nc.dram_tensor("wproj", [HL * D, DIM], f32r, kind="ExternalInput")
    part = nc.dram_tensor("part", [units, N, DIM], f32, kind="ExternalOutput")

    import concourse.bass as bass

    def bcast_part(ap, p=128):
        # replicate a 1D DRAM vector across p partitions (step-0 partition dim)
        return bass.AP(tensor=ap.tensor, offset=ap.offset,
                       ap=[[0, p]] + list(ap.ap))

    with tile.TileContext(nc) as tc:
        import contextlib
        with contextlib.ExitStack() as ctx:
            const = ctx.enter_context(tc.tile_pool(name="const", bufs=1))
            p_x = ctx.enter_context(tc.tile_pool(name="p_x", bufs=2))
            p_qk = ctx.enter_context(tc.tile_pool(name="p_qk", bufs=1))
            p_v = ctx.enter_context(tc.tile_pool(name="p_v", bufs=1))
            p_es = ctx.enter_context(tc.tile_pool(name="p_es", bufs=4))
            p_ot = ctx.enter_context(tc.tile_pool(name="p_ot", bufs=4))
            p_out = ctx.enter_context(tc.tile_pool(name="p_out", bufs=2))
            p_rc = ctx.enter_context(tc.tile_pool(name="p_rc", bufs=8))
            ps_big = ctx.enter_context(
                tc.tile_pool(name="ps_big", bufs=2, space="PSUM"))
            ps_o = ctx.enter_context(
                tc.tile_pool(name="ps_o", bufs=4, space="PSUM"))

            # ---- persistent weights ----
            wq_sb = const.tile([128, 8, 1024 + HL * D], f32r, tag="wqkv")
            _wq_r = wqkv[:].rearrange("(k p) c -> p k c", p=128)
            for k in range(8):
                nc.sync.dma_start(out=wq_sb[:, k, :], in_=_wq_r[:, k, :])
            wp_sb = const.tile([128, 4, DIM], f32r, tag="wproj")
            nc.sync.dma_start(
                out=wp_sb, in_=wproj[:].rearrange("(k p) c -> p k c", p=128))
            bqk_sb = const.tile([128, 8], f32, tag="bqk")
            nc.sync.dma_start(
                out=bqk_sb, in_=bqk[:].rearrange("(c p) -> p c", p=128))
            bv_sb = const.tile([128, HL * VW], f32, tag="bv")
            nc.sync.dma_start(out=bv_sb, in_=bcast_part(bv[:]))
            # persistent zero-padded k^T: per local head lh, kz[:, lh, :] has
            # kT_lh on rows (lh%2)*64..+64 and zeros elsewhere -> score
            # matmuls contract over the full K=128
            kz_sb = const.tile([128, HL, N], f32r, tag="kz")
            for lh in range(HL):
                zp = 64 - (lh % 2) * 64
                nc.vector.tensor_scalar_mul(
                    kz_sb[zp:zp + 64, lh, :], wq_sb[0:64, 0, 0:N], 0.0)

            def unit_body(u):
                x_sb = p_x.tile([128, 8, N], f32r, tag="x")
                nc.sync.dma_start(
                    out=x_sb,
                    in_=xT[0 if "staticx" in phases else u].rearrange(
                        "(k p) n -> p k n", p=128))

                KK = 4 if "halfk" in phases else 8

                def drain(out_ap, in_ap, bias_col=None):
                    # psum -> sbuf on the ACT engine (DVE stays free)
                    if qk_bias and bias_col is not None:
                        nc.scalar.activation(
                            out=out_ap, in_=in_ap, func=AF.Identity,
                            bias=bias_col)
                    else:
                        nc.scalar.activation(
                            out=out_ap, in_=in_ap, func=AF.Copy)

                # ---- qk^T projection: [col, tok] ----
                qk_sb = p_qk.tile([128, 4, N], f32r, tag="qk")
                for ct in range(8):
                    pm = ps_big.tile([128, 2, N], f32, tag="mm")
                    pmh = pm[:, 0, :]
                    for k in range(KK):
                        nc.tensor.matmul(
                            pmh, wq_sb[:, k, ts(ct, 128)], x_sb[:, k, :],
                            start=(k == 0), stop=(k == KK - 1))
                    if ct < 4:  # q columns
                        drain(qk_sb[:, ct, :], pmh, bqk_sb[:, ct:ct + 1])
                    else:  # k columns -> zero-padded per-head kz
                        for half in range(2):
                            hp = half * 64
                            h = 2 * (ct - 4) + half
                            drain(kz_sb[hp:hp + 64, h, :], pmh[hp:hp + 64, :],
                                  bqk_sb[hp:hp + 64, ct:ct + 1])

                # ---- v projection: [tok, lh*65+d]; 65th col per head is
                # the constant 1.0 -> P@V also yields softmax denominators
                v_sb = p_v.tile([128, 4, HL * VW], f32r, tag="v")
                vv = v_sb.rearrange("p t (h w) -> p t h w", w=VW)
                bvv = bv_sb.rearrange("p (h w) -> p h w", w=VW)
                for tt in range(4):
                    pm = ps_big.tile([128, 2, N], f32, tag="mm")
                    pv = pm[:, 0, :]
                    pvv = pv.rearrange("p (h d) -> p h d", d=D)
                    for k in range(KK):
                        nc.tensor.matmul(
                            pv, x_sb[:, k, ts(tt, 128)],
                            wq_sb[:, k, 1024:1024 + HL * D],
                            start=(k == 0), stop=(k == KK - 1))
                    nc.vector.tensor_add(
                        out=vv[:, tt, :, 0:D], in0=pvv, in1=bvv[:, :, 0:D])
                    nc.vector.tensor_scalar(
                        out=vv[:, tt, :, D], in0=bvv[:, :, D],
                        scalar1=0.0, scalar2=1.0,
                        op0=mybir.AluOpType.mult, op1=mybir.AluOpType.add)

                # ---- attention, head-pair batched ----
                # one [128,1024] psum tile + ONE exp per (pair, jt) step;
                # 2-buf psum ring = 2 pair-steps of PE run-ahead over ACT
                nheads = HL if "A" in phases else 0
                steps = [(c, jt) for c in range(nheads // 2)
                         for jt in range(4)]
                pos = {}   # head -> po psum
                ess = {}   # (c, jt) -> es tile
                ots = {}   # pair -> ot tile

                def emit_st(c, jt):
                    pst = ps_big.tile([128, 2, N], f32, tag="mm",
                                      name="pst")
                    for half in range(2):
                        nc.tensor.matmul(
                            pst[:, half, :],
                            kz_sb[:, 2 * c + half, ts(jt, 128)],
                            qk_sb[:, c, :], start=True, stop=True)
                    es_t = p_es.tile([128, 2, N], f32r, tag="es")
                    nc.scalar.activation(out=es_t, in_=pst, func=AF.Exp)
                    ess[(c, jt)] = es_t

                def emit_pav(c, jt):
                    if jt == 0:
                        pos[2 * c] = ps_o.tile([128, N], f32, tag="o",
                                               name=f"po{2 * c}")
                        pos[2 * c + 1] = ps_o.tile([128, N], f32, tag="o",
                                                   name=f"po{2 * c + 1}")
                        ots[c] = p_ot.tile([128, N], f32r, tag="ot",
                                           name=f"ot{c}")
                    es_t = ess.pop((c, jt))
                    for half in range(2):
                        h = 2 * c + half
                        nc.tensor.matmul(
                            pos[h][0:VW, :],
                            v_sb[:, jt, h * VW:(h + 1) * VW],
                            es_t[:, half, :],
                            start=(jt == 0), stop=(jt == 3))
                    if jt == 3:
                        for half in range(2):
                            h = 2 * c + half
                            po = pos.pop(h)
                            rc = p_rc.tile([1, N], f32r, tag="rc")
                            with nc.allow_low_precision(
                                    reason="f32r softmax recip"):
                                nc.vector.reciprocal(
                                    out=rc[0:1, :], in_=po[64:65, :])
                            bc = p_rc.tile([64, N], f32r, tag="bc")
                            nc.gpsimd.partition_broadcast(bc, rc[0:1, :])
                            nc.vector.tensor_mul(
                                out=ots[c][half * 64:half * 64 + 64, :],
                                in0=po[0:64, :], in1=bc)

                LOOKAHEAD = 2
                for s in range(len(steps) + LOOKAHEAD):
                    if s < len(steps):
                        emit_st(*steps[s])
                    if s >= LOOKAHEAD:
                        emit_pav(*steps[s - LOOKAHEAD])

                # ---- partial c_proj: out[tok, e] ----
                if "A" in phases:
                    csrc = [ots[c] for c in range(4)]
                else:
                    csrc = [qk_sb[:, c, :] for c in range(4)]
                if "C" in phases:
                    for tt in range(4):
                        o_sb = p_out.tile([128, DIM], f32, tag="out")
                        for eh in range(2):
                            pm = ps_big.tile([128, 2, N], f32, tag="mm")
                            pc = pm[:, 0, :]
                            for ct in range(4):
                                nc.tensor.matmul(
                                    pc, csrc[ct][:, ts(tt, 128)],
                                    wp_sb[:, ct, eh * 512:(eh + 1) * 512],
                                    start=(ct == 0), stop=(ct == 3))
                            nc.scalar.activation(
                                out=o_sb[:, eh * 512:(eh + 1) * 512],
                                in_=pc, func=AF.Copy)
                        nc.sync.dma_start(
                            out=part[u, ts(tt, 128), :], in_=o_sb)
                else:
                    for tt in range(4):
                        nc.sync.dma_start(
                            out=part[u, ts(tt, 128), :],
                            in_=qk_sb[:, 0:2, :].bitcast(f32))

            if repeat == 1:
                for u in range(units):
                    unit_body(u)
            else:
                with tc.For_i(0, repeat, 1):
                    for u in range(units):
                        unit_body(u)

    nc.compile()
    return nc


def _make_runner(nc, n_cores=NCORES, donate=True):
    """Persistent jitted SPMD runner (mirrors bass2jax.run_bass_via_pjrt)."""
    import jax
    from jax.sharding import Mesh, PartitionSpec
    from jax.experimental.shard_map import shard_map
    from concourse import bass2jax
    from concourse import mybir as mb

    bass2jax.install_neuronx_cc_hook()
    pn = nc.partition_id_tensor.name if nc.partition_id_tensor else None
    in_names, out_names, out_avals, out_shapes = [], [], [], []
    for alloc in nc.m.functions[0].allocations:
        if not isinstance(alloc, mb.MemoryLocationSet):
            continue
        name = alloc.memorylocations[0].name
        if alloc.kind == "ExternalInput":
            if name != pn:
                in_names.append(name)
        elif alloc.kind == "ExternalOutput":
            shape = tuple(alloc.tensor_shape)
            dtype = mb.dt.np(alloc.dtype)
            out_names.append(name)
            out_avals.append(jax.core.ShapedArray(shape, dtype))
            out_shapes.append((shape, dtype))
    n_params = len(in_names)
    n_outs = len(out_names)
    all_in = list(in_names) + list(out_names) + ([pn] if pn else [])

    def _body(*args):
        ops = list(args)
        if pn:
            ops.append(bass2jax.partition_id_tensor())
        return tuple(bass2jax._bass_exec_p.bind(
            *ops, out_avals=tuple(out_avals), in_names=tuple(all_in),
            out_names=tuple(out_names), lowering_input_output_aliases=(),
            sim_require_finite=True, sim_require_nnan=True, nc=nc))

    devices = jax.devices()[:n_cores]
    mesh = Mesh(np.asarray(devices), ("core",))
    specs = (PartitionSpec("core"),)
    fn = jax.jit(
        shard_map(_body, mesh=mesh, in_specs=specs * (n_params + n_outs),
                  out_specs=specs * n_outs, check_rep=False),
        donate_argnums=tuple(range(n_params, n_params + n_outs)) if donate else (),
        keep_unused=True)

    def run(in_maps):
        per_core = [[np.asarray(m[name]) for name in in_names] for m in in_maps]
        concat_in = [np.concatenate([per_core[c][i] for c in range(n_cores)],
                                    axis=0) for i in range(n_params)]
        concat_zeros = [np.zeros((n_cores * s[0], *s[1:]), d)
                        for (s, d) in out_shapes]
        import jax as _jax
        out_arrs = _jax.block_until_ready(fn(*concat_in, *concat_zeros))
        return [
            {name: np.asarray(out_arrs[i]).reshape(n_cores, *out_shapes[i][0])[c]
             for i, name in enumerate(out_names)}
            for c in range(n_cores)
        ]

    run.jit_fn = fn
    run.in_names = in_names
    run.out_names = out_names
    run.out_shapes = out_shapes
    run.n_cores = n_cores
    return run


def _unit_groups():
    units = [(b, r) for b in range(B) for r in range(A)]
    return [units[g * UNITS:(g + 1) * UNITS] for g in range(4)]


def shard_inputs(x, w_qkv, b_qkv, w_proj, b_proj):
    groups = _unit_groups()
    w4 = w_qkv.reshape(DIM, H, 3, D)
    b4 = b_qkv.reshape(H, 3, D)
    in_maps = []
    for c in range(NCORES):
        g, hh = c // 2, c % 2
        heads = list(range(hh * HL, (hh + 1) * HL))
        xT = np.ascontiguousarray(
            np.stack([x[b, :, r, :].T for (b, r) in groups[g]])
        ).astype(np.float32)
        wq = w4[:, heads, 0, :].reshape(DIM, HL * D) * SCALE
        wk = w4[:, heads, 1, :].reshape(DIM, HL * D) * SCALE
        wv = w4[:, heads, 2, :].reshape(DIM, HL * D)
        wqkv_c = np.ascontiguousarray(
            np.concatenate([wq, wk, wv], axis=1)).astype(np.float32)
        bq = (b4[heads, 0, :].reshape(HL * D) * SCALE)
        bk = (b4[heads, 1, :].reshape(HL * D) * SCALE)
        bvv = np.concatenate([b4[heads, 2, :], np.ones((HL, 1), np.float32)],
                             axis=1).reshape(HL * VW)
        in_maps.append({
            "xT": xT,
            "wqkv": wqkv_c,
            "bqk": np.concatenate([bq, bk]).astype(np.float32),
            "bv": bvv.astype(np.float32),
            "wproj": np.ascontiguousarray(
                w_proj[hh * HL * D:(hh + 1) * HL * D, :]).astype(np.float32),
        })
    return in_maps


def unshard(results, b_proj):
    groups = _unit_groups()
    out = np.zeros((B, N, A, DIM), np.float32)
    for g in range(4):
        s = results[2 * g]["part"] + results[2 * g + 1]["part"]
        for idx, (b, r) in enumerate(groups[g]):
            out[b, :, r, :] = s[idx]
    return out + b_proj.astype(np.float32)


def get_runner(qk_bias=False):
    key = ("runner", qk_bias)
    if key not in _CACHE:
        nc = _build_nc(qk_bias=qk_bias)
        _CACHE[key] = _make_runner(nc)
    return _CACHE[key]


def kernel(x, w_qkv, b_qkv, w_proj, b_proj):
    x = np.asarray(x)
    w_qkv = np.asarray(w_qkv)
    b_qkv = np.asarray(b_qkv)
    w_proj = np.asarray(w_proj)
    b_proj = np.asarray(b_proj)
    run = get_runner(qk_bias=bool(np.any(b_qkv[:2048])))
    in_maps = shard_inputs(x, w_qkv, b_qkv, w_proj, b_proj)
    results = run(in_maps)
    return unshard(results, b_proj)


# revision 12
# speedup vs baseline: 1.1556x; 1.1556x over previous
"""Trainium2 Bass kernel for nn_Attention_v4 (sparse per-atom attention).

Reference computation (fp32):
    x:[2,512,14,1024] -> qkv = x@w_qkv+b_qkv -> per (b, r=atom, head) attention
    over the n=512 axis -> out @ w_proj + b_proj.

Sharding (8 cores): 4 groups x 7 (b,r)-units data-parallel, x 2 head-halves
tensor-parallel. Each core computes, for its 7 units and its 8 heads:
QKV^T projection, attention, and a partial c_proj (contraction over its 512
of the 1024 hd rows). Host unshard sums the two head-half partials (the
"all-reduce" of the TP split) and adds b_proj.

Device layouts (all matmuls in float32r: full PE rate at N>=256):
  qkT  [col(q 512|k 512), tok]  - from lhsT=w tiles (stationary), rhs=xT
  v    [tok, lh*65+d]           - 65th col per head = 1.0 (via zero weight
                                  col + bias 1) -> P@V also yields softmax
                                  denominators as row 64 of each head block
  S^T  [j, i] = kT.T @ qT       - softmax over j = partition dim; scores are
                                  O(3) so exp without max-subtraction is safe
  O^T  [hd, i] = (v' @ expS) / denom
  out  [tok, e] partial = sum_ct OT_ct.T @ wproj_ct

v2 pipeline notes (HW-probe driven):
  - scores for a head PAIR share one [128,1024] psum tile and ONE exp, so
    the ACT round-trip is paid once per pair-step and the 2-buf psum ring
    gives ~4 head-steps of PE run-ahead (v1 stalled ~1.4us per head-step).
  - psum: ps_big 2 bufs x 2 banks (qk/v/st/c_proj chunks), ps_o 4 bufs
    (two head-pairs of attention output in flight) = 8 banks exactly.
  - psum drains ride on ACT (Copy/Identity+bias); DVE keeps v-bias adds
    and softmax normalize; GPSIMD only broadcasts the recip denominators.
  - per-pair ot tiles let c_proj's ct-chunk start as soon as that pair is
    normalized instead of waiting for all 8 heads.
"""

import numpy as np

B, N, A, DIM, H, D = 2, 512, 14, 1024, 16, 64
HL = 8            # heads per core
UNITS = 7         # (b, r) units per group
NCORES = 8
SCALE = np.float32(1.0 / np.sqrt(np.sqrt(D)))
VW = D + 1        # v width per head incl. ones column

_CACHE = {}


def _build_nc(units=UNITS, repeat=1, phases="QAC", qk_bias=False):
    import itertools
    import concourse.bacc as bacc
    import concourse.tile as tile
    from concourse import mybir
    from concourse.bass import ts

    f32, f32r = mybir.dt.float32, mybir.dt.float32r
    AF = mybir.ActivationFunctionType

    nc = bacc.Bacc("TRN2", target_bir_lowering=False, debug=False,
                   num_devices=NCORES)
    xT = nc.dram_tensor("xT", [units, DIM, N], f32r, kind="ExternalInput")
    wqkv = nc.dram_tensor("wqkv", [DIM, 1024 + HL * D], f32r,
                          kind="ExternalInput")
    bqk = nc.dram_tensor("bqk", [1024], f32, kind="ExternalInput")
    bv = nc.dram_tensor("bv", [HL * VW], f32, kind="ExternalInput")
    wproj = nc.dram_tensor("wproj", [HL * D, DIM], f32r, kind="ExternalInput")
    part = nc.dram_tensor("part", [units, N, DIM], f32, kind="ExternalOutput")

    import concourse.bass as bass

    def bcast_part(ap, p=128):
        # replicate a 1D DRAM vector across p partitions (step-0 partition dim)
        return bass.AP(tensor=ap.tensor, offset=ap.offset,
                       ap=[[0, p]] + list(ap.ap))

    # ot buffer colors: consecutive units must differ (att(u) writes color
    # c(u) while cproj(u-1), woven into att(u)'s stream, reads c(u-1));
    # with an odd unit count the wrap-around pair needs a third color.
    color = [i % 2 for i in range(units)]
    if units % 2 == 1 and units > 1:
        color[-1] = 2
    ncolor = max(color) + 1

    with tile.TileContext(nc) as tc:
        import contextlib
        with contextlib.ExitStack() as ctx:
            const = ctx.enter_context(tc.tile_pool(name="const", bufs=1))
            p_x = ctx.enter_context(tc.tile_pool(name="p_x", bufs=2))
            p_qk = ctx.enter_context(tc.tile_pool(name="p_qk", bufs=2))
            p_kz = ctx.enter_context(tc.tile_pool(name="p_kz", bufs=2))
            p_v = ctx.enter_context(tc.tile_pool(name="p_v", bufs=2))
            p_es = ctx.enter_context(tc.tile_pool(name="p_es", bufs=3))
            p_out = ctx.enter_context(tc.tile_pool(name="p_out", bufs=2))
            p_rc = ctx.enter_context(tc.tile_pool(name="p_rc", bufs=2))
            p_bc = ctx.enter_context(tc.tile_pool(name="p_bc", bufs=3))
            ps_big = ctx.enter_context(
                tc.tile_pool(name="ps_big", bufs=2, space="PSUM"))
            ps_o = ctx.enter_context(
                tc.tile_pool(name="ps_o", bufs=2, space="PSUM"))

            # ---- persistent weights ----
            wq_sb = const.tile([128, 8, 1024 + HL * D], f32r, tag="wqkv")
            _wq_r = wqkv[:].rearrange("(k p) c -> p k c", p=128)
            for k in range(8):
                nc.sync.dma_start(out=wq_sb[:, k, :], in_=_wq_r[:, k, :])
            wp_sb = const.tile([128, 4, DIM], f32r, tag="wproj")
            nc.sync.dma_start(
                out=wp_sb, in_=wproj[:].rearrange("(k p) c -> p k c", p=128))
            bqk_sb = const.tile([128, 8], f32, tag="bqk")
            nc.sync.dma_start(
                out=bqk_sb, in_=bqk[:].rearrange("(c p) -> p c", p=128))
            bv_sb = const.tile([128, HL * VW], f32, tag="bv")
            nc.sync.dma_start(out=bv_sb, in_=bcast_part(bv[:]))
            # attention-output buffers, statically 3-colored (see above)
            ot_cols = [const.tile([128, 4, N], f32r, tag=f"otc{i}",
                                  name=f"otc{i}") for i in range(ncolor)]

            bvv = bv_sb.rearrange("p (h w) -> p h w", w=VW)
            KK = 4 if "halfk" in phases else 8
            store = {}  # u -> (qk_sb, kz_sb, v_sb)

            def proj_gen(u):
                """qkv projection of unit u: 1 setup + 12 chunk slots.

                q lands in qk_sb [128cols(2 heads), chunk, tok]; k lands
                PACKED the same way in kz_sb (score matmuls contract K=64
                from base partition 0/64); v in v_sb [tok, head*65+d] with
                the 65th ones-column riding for softmax denominators.
                """
                x_sb = p_x.tile([128, 8, N], f32r, tag="x", name="x_sb")
                nc.sync.dma_start(
                    out=x_sb,
                    in_=xT[0 if "staticx" in phases else u].rearrange(
                        "(k p) n -> p k n", p=128))
                qk_sb = p_qk.tile([128, 4, N], f32r, tag="qk", name="qk_sb")
                kz_sb = p_kz.tile([128, 4, N], f32r, tag="kz", name="kz_sb")
                v_sb = p_v.tile([128, 4, HL * VW], f32r, tag="v", name="v_sb")
                store[u] = (qk_sb, kz_sb, v_sb)
                yield
                for ct in range(8):
                    pm = ps_big.tile([128, 2, N], f32, tag="mm", name="pm")
                    pmh = pm[:, 0, :]
                    for k in range(KK):
                        nc.tensor.matmul(
                            pmh, wq_sb[:, k, ts(ct, 128)], x_sb[:, k, :],
                            start=(k == 0), stop=(k == KK - 1))
                    dst = qk_sb[:, ct, :] if ct < 4 else kz_sb[:, ct - 4, :]
                    if ct < 4:  # q -> ACT drain (DVE stays free)
                        if qk_bias:
                            nc.scalar.activation(
                                out=dst, in_=pmh, func=AF.Identity,
                                bias=bqk_sb[:, ct:ct + 1])
                        else:
                            nc.scalar.activation(
                                out=dst, in_=pmh, func=AF.Copy)
                    else:  # k -> DVE drain
                        if qk_bias:
                            nc.vector.tensor_scalar_add(
                                dst, pmh, bqk_sb[:, ct:ct + 1])
                        else:
                            nc.vector.tensor_copy(out=dst, in_=pmh)
                    yield
                for tt in range(4):
                    pm = ps_big.tile([128, 2, N], f32, tag="mm", name="pv")
                    pv = pm[:, 0, :]
                    pvv = pv.rearrange("p (h d) -> p h d", d=D)
                    vv = v_sb.rearrange("p t (h w) -> p t h w", w=VW)
                    for k in range(KK):
                        nc.tensor.matmul(
                            pv, x_sb[:, k, ts(tt, 128)],
                            wq_sb[:, k, 1024:1024 + HL * D],
                            start=(k == 0), stop=(k == KK - 1))
                    nc.vector.tensor_add(
                        out=vv[:, tt, :, 0:D], in0=pvv, in1=bvv[:, :, 0:D])
                    nc.vector.tensor_scalar(
                        out=vv[:, tt, :, D], in0=bvv[:, :, D],
                        scalar1=0.0, scalar2=1.0,
                        op0=mybir.AluOpType.mult, op1=mybir.AluOpType.add)
                    yield

            def att_gen(u):
                """attention of unit u: 16 pair-steps + LOOKAHEAD slots.

                Per (head-pair, j-chunk): two K=64 score matmuls into one
                2-bank psum tile, ONE exp for both heads, then (LOOKAHEAD
                slots later) two accumulating PAV matmuls.  The pair's po
                psum tile carries the softmax denominators in row 64 of
                each half; normalize = recip + gpsimd broadcast + DVE mul
                into the unit's color'd ot buffer.
                """
                if "A" not in phases:
                    return
                qk_sb, kz_sb, v_sb = store[u]
                otc = ot_cols[color[u]]
                ess = {}
                pos = {}

                def emit_st(c, jt):
                    pst = ps_big.tile([128, 2, N], f32, tag="mm", name="pst")
                    for half in range(2):
                        hp = half * 64
                        nc.tensor.matmul(
                            pst[:, half, :],
                            kz_sb[hp:hp + 64, c, ts(jt, 128)],
                            qk_sb[hp:hp + 64, c, :], start=True, stop=True)
                    es_t = p_es.tile([128, 2, N], f32r, tag="es",
                                     name="es_t")
                    nc.scalar.activation(out=es_t, in_=pst, func=AF.Exp)
                    ess[(c, jt)] = es_t

                def emit_pav(c, jt):
                    if "nopav" in phases:
                        ess.pop((c, jt))
                        return
                    if jt == 0:
                        pos[c] = ps_o.tile([128, 2, N], f32, tag="o",
                                           name=f"po{c}")
                    po = pos[c]
                    es_t = ess.pop((c, jt))
                    for half in range(2):
                        h = 2 * c + half
                        nc.tensor.matmul(
                            po[0:VW, half, :],
                            v_sb[:, jt, h * VW:(h + 1) * VW],
                            es_t[:, half, :],
                            start=(jt == 0), stop=(jt == 3))
                    if jt == 3:
                        pos.pop(c)
                        if "nonorm" in phases:  # timing probe only
                            for half in range(2):
                                nc.scalar.activation(
                                    out=otc[half * 64:half * 64 + 64, c, :],
                                    in_=po[0:64, half, :], func=AF.Copy)
                            return
                        rc = p_rc.tile([1, 2, N], f32r, tag="rc", name="rc")
                        with nc.allow_low_precision(
                                reason="f32r softmax recip"):
                            nc.vector.reciprocal(
                                out=rc, in_=po[64:65, :, :])
                        for half in range(2):
                            bc = p_bc.tile([64, N], f32r, tag="bc",
                                           name="bc")
                            nc.gpsimd.partition_broadcast(
                                bc, rc[0:1, half, :])
                            nc.vector.tensor_mul(
                                out=otc[half * 64:half * 64 + 64, c, :],
                                in0=po[0:64, half, :], in1=bc)

                steps = [(c, jt) for c in range(4) for jt in range(4)]
                LOOKAHEAD = 2
                for s in range(len(steps) + LOOKAHEAD):
                    if s < len(steps):
                        emit_st(*steps[s])
                    if s >= LOOKAHEAD:
                        emit_pav(*steps[s - LOOKAHEAD])
                    yield

            def cproj_gen(u):
                """partial c_proj of unit u (4 token-chunk slots)."""
                if "C" not in phases:
                    if "A" not in phases and u in store:
                        qk_sb = store[u][0]
                        for tt in range(4):
                            nc.sync.dma_start(
                                out=part[u, ts(tt, 128), :],
                                in_=qk_sb[:, 0:2, :].bitcast(f32))
                            yield
                    return
                otc = ot_cols[color[u]]
                for tt in range(4):
                    o_sb = p_out.tile([128, DIM], f32, tag="out",
                                      name="o_sb")
                    for eh in range(2):
                        pm = ps_big.tile([128, 2, N], f32, tag="mm",
                                         name="pc")
                        pc = pm[:, 0, :]
                        for ct in range(4):
                            nc.tensor.matmul(
                                pc, otc[:, ct, ts(tt, 128)],
                                wp_sb[:, ct, eh * 512:(eh + 1) * 512],
                                start=(ct == 0), stop=(ct == 3))
                        nc.vector.tensor_copy(
                            out=o_sb[:, eh * 512:(eh + 1) * 512], in_=pc)
                    nc.sync.dma_start(
                        out=part[u, ts(tt, 128), :], in_=o_sb)
                    yield

            def weave(u, emit_cproj, emit_proj):
                """attention(u) interleaved with cproj(u-1) + proj(u+1):
                the PE stream never waits on the exp round trip because
                projection/c_proj chunks fill the windows, and the ACT-
                bound attention overlaps PE-bound projection work."""
                fills = []
                if emit_proj:
                    pg = proj_gen((u + 1) % units)
                    next(pg)  # allocate tiles + kick x DMA immediately
                    fills.append(pg)
                if emit_cproj:
                    fills.insert(0, cproj_gen((u - 1) % units))
                fill = itertools.chain(*fills)
                nfill = (12 if emit_proj else 0) + (4 if emit_cproj else 0)
                natt = 18 if "A" in phases else 0
                if natt == 0:
                    for _ in fill:
                        pass
                    return
                fi = 0
                s = 0
                for _ in att_gen(u):
                    s += 1
                    want = (s * nfill) // natt
                    while fi < want:
                        try:
                            next(fill)
                            fi += 1
                        except StopIteration:
                            fi = nfill
                            break
                for _ in fill:
                    pass

            def emit_all(in_loop):
                # proj(0) stands alone at body start so every tile read by
                # the body is allocated within the body (a prologue-bound
                # read would point at ring buffers later reused in-body and
                # deadlock the scheduler on the loop-back edge).
                for _ in proj_gen(0):
                    pass
                for u in range(units):
                    emit_cproj = (u >= 1 or in_loop) and units > 1
                    emit_proj = (u < units - 1) and units > 1
                    weave(u, emit_cproj, emit_proj)

            if units == 1:
                for _ in proj_gen(0):
                    pass
                for _ in att_gen(0):
                    pass
                for _ in cproj_gen(0):
                    pass
            elif repeat == 1:
                emit_all(in_loop=False)
                for _ in cproj_gen(units - 1):
                    pass
            else:
                with tc.For_i(0, repeat, 1):
                    emit_all(in_loop=True)
                for _ in cproj_gen(units - 1):
                    pass

    nc.compile()
    return nc


def _make_runner(nc, n_cores=NCORES, donate=True):
    """Persistent jitted SPMD runner (mirrors bass2jax.run_bass_via_pjrt)."""
    import jax
    from jax.sharding import Mesh, PartitionSpec
    from jax.experimental.shard_map import shard_map
    from concourse import bass2jax
    from concourse import mybir as mb

    bass2jax.install_neuronx_cc_hook()
    pn = nc.partition_id_tensor.name if nc.partition_id_tensor else None
    in_names, out_names, out_avals, out_shapes = [], [], [], []
    for alloc in nc.m.functions[0].allocations:
        if not isinstance(alloc, mb.MemoryLocationSet):
            continue
        name = alloc.memorylocations[0].name
        if alloc.kind == "ExternalInput":
            if name != pn:
                in_names.append(name)
        elif alloc.kind == "ExternalOutput":
            shape = tuple(alloc.tensor_shape)
            dtype = mb.dt.np(alloc.dtype)
            out_names.append(name)
            out_avals.append(jax.core.ShapedArray(shape, dtype))
            out_shapes.append((shape, dtype))
    n_params = len(in_names)
    n_outs = len(out_names)
    all_in = list(in_names) + list(out_names) + ([pn] if pn else [])

    def _body(*args):
        ops = list(args)
        if pn:
            ops.append(bass2jax.partition_id_tensor())
        return tuple(bass2jax._bass_exec_p.bind(
            *ops, out_avals=tuple(out_avals), in_names=tuple(all_in),
            out_names=tuple(out_names), lowering_input_output_aliases=(),
            sim_require_finite=True, sim_require_nnan=True, nc=nc))

    devices = jax.devices()[:n_cores]
    mesh = Mesh(np.asarray(devices), ("core",))
    specs = (PartitionSpec("core"),)
    fn = jax.jit(
        shard_map(_body, mesh=mesh, in_specs=specs * (n_params + n_outs),
                  out_specs=specs * n_outs, check_rep=False),
        donate_argnums=tuple(range(n_params, n_params + n_outs)) if donate else (),
        keep_unused=True)

    def run(in_maps):
        per_core = [[np.asarray(m[name]) for name in in_names] for m in in_maps]
        concat_in = [np.concatenate([per_core[c][i] for c in range(n_cores)],
                                    axis=0) for i in range(n_params)]
        concat_zeros = [np.zeros((n_cores * s[0], *s[1:]), d)
                        for (s, d) in out_shapes]
        import jax as _jax
        out_arrs = _jax.block_until_ready(fn(*concat_in, *concat_zeros))
        return [
            {name: np.asarray(out_arrs[i]).reshape(n_cores, *out_shapes[i][0])[c]
             for i, name in enumerate(out_names)}
            for c in range(n_cores)
        ]

    run.jit_fn = fn
    run.in_names = in_names
    run.out_names = out_names
    run.out_shapes = out_shapes
    run.n_cores = n_cores
    return run


def _unit_groups():
    units = [(b, r) for b in range(B) for r in range(A)]
    return [units[g * UNITS:(g + 1) * UNITS] for g in range(4)]


def shard_inputs(x, w_qkv, b_qkv, w_proj, b_proj):
    groups = _unit_groups()
    w4 = w_qkv.reshape(DIM, H, 3, D)
    b4 = b_qkv.reshape(H, 3, D)
    in_maps = []
    for c in range(NCORES):
        g, hh = c // 2, c % 2
        heads = list(range(hh * HL, (hh + 1) * HL))
        xT = np.ascontiguousarray(
            np.stack([x[b, :, r, :].T for (b, r) in groups[g]])
        ).astype(np.float32)
        wq = w4[:, heads, 0, :].reshape(DIM, HL * D) * SCALE
        wk = w4[:, heads, 1, :].reshape(DIM, HL * D) * SCALE
        wv = w4[:, heads, 2, :].reshape(DIM, HL * D)
        wqkv_c = np.ascontiguousarray(
            np.concatenate([wq, wk, wv], axis=1)).astype(np.float32)
        bq = (b4[heads, 0, :].reshape(HL * D) * SCALE)
        bk = (b4[heads, 1, :].reshape(HL * D) * SCALE)
        bvv = np.concatenate([b4[heads, 2, :], np.ones((HL, 1), np.float32)],
                             axis=1).reshape(HL * VW)
        in_maps.append({
            "xT": xT,
            "wqkv": wqkv_c,
            "bqk": np.concatenate([bq, bk]).astype(np.float32),
            "bv": bvv.astype(np.float32),
            "wproj": np.ascontiguousarray(
                w_proj[hh * HL * D:(hh + 1) * HL * D, :]).astype(np.float32),
        })
    return in_maps


def unshard(results, b_proj):
    groups = _unit_groups()
    out = np.zeros((B, N, A, DIM), np.float32)
    for g in range(4):
        s = results[2 * g]["part"] + results[2 * g + 1]["part"]
        for idx, (b, r) in enumerate(groups[g]):
            out[b, :, r, :] = s[idx]
    return out + b_proj.astype(np.float32)


def get_runner(qk_bias=False):
    key = ("runner", qk_bias)
    if key not in _CACHE:
        nc = _build_nc(qk_bias=qk_bias)
        _CACHE[key] = _make_runner(nc)
    return _CACHE[key]


def kernel(x, w_qkv, b_qkv, w_proj, b_proj):
    x = np.asarray(x)
    w_qkv = np.asarray(w_qkv)
    b_qkv = np.asarray(b_qkv)
    w_proj = np.asarray(w_proj)
    b_proj = np.asarray(b_proj)
    run = get_runner(qk_bias=bool(np.any(b_qkv[:2048])))
    in_maps = shard_inputs(x, w_qkv, b_qkv, w_proj, b_proj)
    results = run(in_maps)
    return unshard(results, b_proj)
